# revision 32
# baseline (speedup 1.0000x reference)
"""Trainium2 Bass kernel for the AttnModel problem.

Pure data-parallel: batch B=1024 sharded as 128 per core across 8 cores,
small parameters replicated. Inside each core:

  - k = [seq | seq_e | seq_t] tiles are streamed in natural [n, d] layout
    and cast fp32->bf16 during the SWDGE (gpsimd) DMA: HBM reads are the
    mandatory 100.7MB fp32, SBUF writes/footprint halve, and bf16 unlocks
    DVE 2x mode and full-rate PE streaming.
  - sk[n] = k[n,:] . wk computed with fused multiply+reduce on VectorE
    (softmax over n is shift-invariant, so the q-score sq cancels and is
    never computed). Softmax itself stays fp32.
  - softmax is batched over sub-blocks of SB batches using PE transposes
    (partition-dim reductions are not available on VectorE).
  - o = attn @ k on the TensorEngine with the attention column as the
    1-wide STATIONARY operand and k streaming 384-wide: 4 matmuls per
    batch instead of 12 width-1 matmuls with 128-col weight reloads.
    (Width-1 matmuls measured ~436ns each on HW -- cold/throttled PE and
    per-instruction overhead made the old scheme ~670us of the 1022us
    total.)  Each batch's [1, 768] PSUM row (bank-aligned 2x512 regions;
    a matmul out must not straddle a 2KB PSUM bank) is evacuated by an
    Act copy to a partition-0 staging row, then a 3KB SBUF->SBUF DMA
    scatters it to the batch's partition in o_rows (engines cannot write
    SBUF at arbitrary base partitions; DMA can).
  - o_rows is transposed to feature-major oT once per 128 batches; the
    dense chain (fc -> +q residual -> LayerNorm -> agg1+relu -> agg2)
    stays feature-major fp32 with pre-transposed weights; LN statistics
    over the feature (partition) dim are computed with ones-vector
    matmuls; the q residual is folded into the fc accumulation as
    transpose-matmuls.

Pipelining (tuned against the TimelineSim cost model + HW measurements):
  - k tiles are half-split per sub-block (2 tags x 2 bufs) so the DMA
    stream runs continuously ~2 sub-blocks ahead of compute; k DMAs ride
    the gpsimd SWDGE queue, mask/attn_w/scatter DMAs the SP HWDGE queue.
  - Weight prep (loads + PE transposes, Act evictions) is emitted BEFORE
    phase A: its DMAs ride the otherwise-idle SP HWDGE queue under the k
    stream, so a single (cold) run has no serial weight-prep tail.
  - Every scratch tile a VectorE op writes has >=2 buffers (a single slot
    creates a WAW chain that serializes the engine stream).
  - Accuracy: bf16 k/attn with fp32 softmax + fp32 PSUM accumulation and
    fp32 dense chain -> rel err ~1.6e-4 (out) / ~4.3e-3 (attn_w) vs the
    fp32 reference, comfortably under the 2e-2 gate.
"""

import numpy as np
from contextlib import ExitStack

import concourse.bass as bass
import concourse.tile as tile
from concourse import bacc, mybir
from concourse.bass_utils import run_bass_kernel_spmd
from concourse.masks import make_identity

F32 = mybir.dt.float32
BF16 = mybir.dt.bfloat16
U8 = mybir.dt.uint8
AF = mybir.ActivationFunctionType
ALU = mybir.AluOpType
AX = mybir.AxisListType

B, N, D = 1024, 256, 256
M = 3 * D
NCORES = 8
NEG = -1e10
LN_EPS = 1e-5

NCH = N // 128   # n chunks (2)
MCH = M // 128   # m chunks (6)
DCH = D // 128   # d chunks (2)
FCH = (M + D) // 128  # agg1 input chunks (8)
OCH = D // 128   # output chunks (2)


def build_bass(bpc=B // NCORES, sb=8, gps_frac=0, stage=4, chain_tok=False,
               repeat=1, kbufs=2, smbufs=5, tpbufs=2, taper=False,
               kdt="bf16"):
    """Build the per-core Bass module. bpc = batches per core.

    stage (debug): 1=sk+softmax only, 2=+attn@k, 3=+fc+LN, 4=full.
    chain_tok: add a tok->tok_out passthrough (for serialized timing)."""
    nsb = bpc // sb
    assert nsb * sb == bpc
    bp = bpc  # partition count for batch-major tiles

    nc = bacc.Bacc()
    seq_h = nc.declare_dram_parameter("seq", [bpc, N, D], F32, isOutput=False)
    seqe_h = nc.declare_dram_parameter("seq_e", [bpc, N, D], F32, isOutput=False)
    seqt_h = nc.declare_dram_parameter("seq_t", [bpc, N, D], F32, isOutput=False)
    src_h = nc.declare_dram_parameter("src", [bpc, D], F32, isOutput=False)
    srct_h = nc.declare_dram_parameter("src_t", [bpc, 1, D], F32, isOutput=False)
    mask_h = nc.declare_dram_parameter("mask", [bpc, N], U8, isOutput=False)
    sha_h = nc.declare_dram_parameter("shared_attn", [1, 2 * M], F32, isOutput=False)
    fcw_h = nc.declare_dram_parameter("fc_w", [M, M], F32, isOutput=False)
    lnw_h = nc.declare_dram_parameter("ln_w", [M], F32, isOutput=False)
    lnb_h = nc.declare_dram_parameter("ln_b", [M], F32, isOutput=False)
    w1_h = nc.declare_dram_parameter("agg_fc_w1", [M, M + D], F32, isOutput=False)
    w2_h = nc.declare_dram_parameter("agg_fc_w2", [D, M], F32, isOutput=False)
    out_h = nc.declare_dram_parameter("out", [bpc, D], F32, isOutput=True)
    attnw_h = nc.declare_dram_parameter("attn_w", [bpc, N], F32, isOutput=True)
    tok_h = tokout_h = None
    if chain_tok:
        tok_h = nc.declare_dram_parameter("tok", [128, 128], F32, isOutput=False)
        tokout_h = nc.declare_dram_parameter("tok_out", [128, 128], F32,
                                             isOutput=True)

    with ExitStack() as ctx:
        tc = ctx.enter_context(tile.TileContext(nc))
        const = ctx.enter_context(tc.tile_pool(name="const", bufs=1))
        wnat = ctx.enter_context(tc.tile_pool(name="wnat", bufs=3))
        kpool = ctx.enter_context(tc.tile_pool(name="kpool", bufs=kbufs))
        jpool = ctx.enter_context(tc.tile_pool(name="jpool", bufs=2))
        sm = ctx.enter_context(tc.tile_pool(name="sm", bufs=smbufs))
        pb = ctx.enter_context(tc.tile_pool(name="pb", bufs=1))
        tp = ctx.enter_context(tc.tile_pool(name="tp", bufs=tpbufs, space="PSUM"))
        bigp = ctx.enter_context(tc.tile_pool(name="bigp", bufs=1, space="PSUM"))
        obuf = ctx.enter_context(tc.tile_pool(name="obuf", bufs=2, space="PSUM"))

        # ---------------- constants / weight prep ----------------
        identity = const.tile([128, 128], F32)
        make_identity(nc, identity)

        # k dtype: bf16 k tiles are cast during the SWDGE DMA; sk accum and
        # softmax stay fp32, so only the k values and attn weights round.
        k_dt = BF16 if kdt == "bf16" else F32
        k_dma = nc.gpsimd if kdt == "bf16" else nc.sync

        # wk broadcast to all partitions: [128, M]
        wk_bcast = const.tile([128, M], k_dt)
        wk_ap = sha_h[0, M:2 * M]
        nc.gpsimd.dma_start(
            out=wk_bcast,
            in_=bass.AP(tensor=wk_ap.tensor, offset=wk_ap.offset,
                        ap=[[0, 128]] + [list(a) for a in wk_ap.ap]),
        )

        ones_col = const.tile([128, 1], F32)
        nc.vector.memset(ones_col, 1.0)
        ones_row = const.tile([1, 128], F32)
        nc.vector.memset(ones_row, 1.0)
        eps_t = const.tile([1, 1], F32)
        nc.vector.memset(eps_t, LN_EPS)

        if chain_tok:
            tok_t = const.tile([128, 128], F32)
            nc.sync.dma_start(out=tok_t, in_=tok_h[:, :])
            nc.sync.dma_start(out=tokout_h[:, :], in_=tok_t)
        for _rep in range(repeat):
            # o_rows[b, m]: attention outputs in batch-major SBUF rows,
            # filled one partition-row per batch by Activation-engine
            # evacuations of the per-batch PSUM matmul results
            o_rows = pb.tile([bp, M], F32, tag="orows", bufs=1)

            # early weight prep: k rides the gpsimd SWDGE queue, so weight
            # DMAs on the SP HWDGE queue no longer contend with the k
            # stream; emitting them first overlaps the prep under phase A
            # instead of exposing it as a serial tail before phase B.
            # (PSUM evictions go through Act, keeping DVE free for sk.)
            if stage >= 1 and _rep == 0:
                lnw_cols = const.tile([128, MCH], F32)
                nc.gpsimd.dma_start(out=lnw_cols, in_=lnw_h[:].rearrange("(c p) -> p c", p=128))
                lnb_cols = const.tile([128, MCH], F32)
                nc.gpsimd.dma_start(out=lnb_cols, in_=lnb_h[:].rearrange("(c p) -> p c", p=128))

                def load_transposed(w_h, rows, cols, name):
                    """w_h: [rows, cols] natural. Returns per-column-chunk tiles
                    [128_col, rows] (i.e. the transposed weight)."""
                    rch, cch = rows // 128, cols // 128
                    wT = [const.tile([128, rows], F32, name=f"{name}T{cc}")
                          for cc in range(cch)]
                    for rc in range(rch):
                        wn = wnat.tile([128, cols], F32, name=f"{name}nat", tag="wnat")
                        nc.sync.dma_start(out=wn, in_=w_h[rc * 128:(rc + 1) * 128, :])
                        for cc in range(cch):
                            pt = tp.tile([128, 128], F32, name="wtp", tag="tp")
                            nc.tensor.transpose(pt, wn[:, cc * 128:(cc + 1) * 128], identity)
                            nc.scalar.copy(wT[cc][:, rc * 128:(rc + 1) * 128], pt)
                    return wT

                fc_wT = load_transposed(fcw_h, M, M, "fcw")      # 6 x [128_m, 768_j]
                w1T = load_transposed(w1_h, M, M + D, "w1")      # 8 x [128_f, 768_i]
                w2T = load_transposed(w2_h, D, M, "w2")          # 6 x [128_i, 256_o]

                src_rows = const.tile([bp, D], F32)
                nc.sync.dma_start(out=src_rows, in_=src_h[:, :])
                srct_rows = const.tile([bp, D], F32)
                nc.sync.dma_start(out=srct_rows, in_=srct_h[:, 0, :])

                # srcT [128_d, DCH, bp] for the agg1 concat part
                srcT = const.tile([128, DCH, bp], F32)
                for dc in range(DCH):
                    pt = tp.tile([128, bp], F32, name="srctp", tag="tp")
                    nc.tensor.transpose(pt, src_rows[:, dc * 128:(dc + 1) * 128],
                                        identity[:bp, :bp])
                    nc.scalar.copy(srcT[:, dc, :], pt)

            # ---------------- phase A: attention ----------------
            # taper: run the last sub-block as two half-size ones so the
            # final DVE+softmax drain after the last DMA is shorter
            schedule = [(i * sb, sb) for i in range(nsb)]
            if taper and sb % 2 == 0 and nsb >= 2:
                l0 = (nsb - 1) * sb
                schedule = ([(0, sb // 2), (sb // 2, sb // 2)]
                            + schedule[1:-1]
                            + [(l0, sb // 2), (l0 + sb // 2, sb // 2)])
            for s, (b0, sbs) in enumerate(schedule):
                # two half-tiles per sub-block: the second half's slot can be
                # refilled while the first half's matmuls are still draining
                hb = sbs // 2
                k_hv = [kpool.tile([128, hb, NCH, 3, D], k_dt,
                                   name=f"k_h{hh}", tag=f"k_h{hh}")
                        for hh in range(2)]
                for hh in range(2):
                    for t, h in enumerate([seq_h, seqe_h, seqt_h]):
                        k_dma.dma_start(
                            out=k_hv[hh][:, :, :, t, :],
                            in_=h[b0 + hh * hb:b0 + (hh + 1) * hb].rearrange(
                                "sbs (c p) d -> p sbs c d", p=128),
                        )
                def k_at(bi):
                    return k_hv[bi // hb][:, bi % hb]
                if stage < 1:
                    # probe mode: tiny consumer so the k DMAs can't be pruned
                    probe = sm.tile([128, 2], F32, tag="probe")
                    nc.vector.tensor_tensor(probe, k_hv[0][:, 0, 0, 0, 0:2],
                                            k_hv[1][:, 0, 0, 0, 0:2], ALU.add)
                    nc.sync.dma_start(out=attnw_h[b0:b0 + 1, 0:2],
                                      in_=probe[0:1, :])
                    continue
                mask_s = sm.tile([sbs, N], U8, tag="mask_s")
                nc.sync.dma_start(out=mask_s, in_=mask_h[b0:b0 + sbs, :])
                m_neg = sm.tile([sbs, N], F32, tag="m_neg")
                nc.vector.tensor_scalar(m_neg, mask_s, float(NEG), None, ALU.mult)

                # sk[n] = k[n, :] . wk   (fused multiply + free-dim reduce)
                sk_st = [sm.tile([128, sbs], F32, name=f"sk{c}", tag=f"sk{c}")
                         for c in range(NCH)]
                opid = 0
                for bi in range(sbs):
                    for c in range(NCH):
                        k_flat = k_at(bi)[:, c].rearrange("p t d -> p (t d)")
                        acc = sk_st[c][:, bi:bi + 1]
                        if gps_frac and opid % gps_frac == gps_frac - 1:
                            # offload: multiply on GpSimd, reduce on ScalarE
                            jg = jpool.tile([128, M], F32, name="junk_g", tag="junk_g")
                            nc.gpsimd.tensor_mul(jg, k_flat, wk_bcast)
                            jg2 = jpool.tile([128, M], F32, name="junk_g2",
                                             tag="junk_g2")
                            nc.scalar.activation(out=jg2, in_=jg, func=AF.Copy,
                                                 accum_out=acc)
                        else:
                            # fused multiply + free-dim reduce on VectorE
                            jv = jpool.tile([128, M], k_dt, name="junk_v", tag="junk_v")
                            nc.vector.scalar_tensor_tensor(
                                out=jv, in0=k_flat, scalar=1.0, in1=wk_bcast,
                                op0=ALU.mult, op1=ALU.mult, accum_out=acc)
                        opid += 1

                # transpose sk to batch-major rows, add mask penalty
                s_rows = sm.tile([sbs, N], F32, tag="s_rows")
                for c in range(NCH):
                    pt = tp.tile([sbs, 128], F32, name="sktp", tag="tp")
                    nc.tensor.transpose(pt, sk_st[c], identity)
                    nc.vector.scalar_tensor_tensor(
                        out=s_rows[:, c * 128:(c + 1) * 128], in0=pt, scalar=1.0,
                        in1=m_neg[:, c * 128:(c + 1) * 128],
                        op0=ALU.mult, op1=ALU.add)

                # softmax over free dim
                nrmax = sm.tile([sbs, 1], F32, tag="nrmax")
                nc.vector.tensor_reduce(out=nrmax, in_=s_rows, axis=AX.X,
                                        op=ALU.max, negate=True)
                p_rows = sm.tile([sbs, N], F32, tag="p_rows")
                rsum = sm.tile([sbs, 1], F32, tag="rsum")
                nc.scalar.activation(out=p_rows, in_=s_rows, func=AF.Exp,
                                     bias=nrmax[:, 0:1], scale=1.0, accum_out=rsum)
                rinv = sm.tile([sbs, 1], F32, tag="rinv")
                nc.vector.reciprocal(rinv, rsum)
                a_rows = sm.tile([sbs, N], F32, tag="a_rows")
                nc.vector.tensor_scalar(a_rows, p_rows, rinv[:, 0:1], None, ALU.mult)
                nc.sync.dma_start(out=attnw_h[b0:b0 + sbs, :], in_=a_rows)

                # attn back to n-major columns for the PE contraction
                attnT = sm.tile([128, NCH, sbs], k_dt, tag="attnT")
                for c in range(NCH):
                    pt2 = tp.tile([128, sbs], F32, name="attp", tag="tp")
                    nc.tensor.transpose(pt2, a_rows[:, c * 128:(c + 1) * 128],
                                        identity[:sbs, :sbs])
                    nc.scalar.copy(attnT[:, c, :], pt2)

                if stage < 2:
                    continue
                # o[b, :] = sum_n attn[n] * k[n, :] with the attention column
                # as the 1-wide stationary operand and k streaming 384-wide:
                # 4 matmuls per batch (2 n-chunks x 2 out-halves) into a
                # [1, 2, 384] PSUM row, then one Act copy into o_rows[b].
                for bi in range(sbs):
                    col = b0 + bi
                    k_row = k_at(bi).rearrange("p c t d -> p c (t d)")
                    # [1, 2, 512]: each 384-wide matmul region starts on a
                    # 2KB PSUM bank boundary (a matmul out must not straddle
                    # banks)
                    ob = obuf.tile([1, 2, 512], F32, tag="ob")
                    for c in range(NCH):
                        for cc in range(2):
                            nc.tensor.matmul(
                                ob[0:1, cc, 0:384],
                                lhsT=attnT[:, c, bi:bi + 1],
                                rhs=k_row[:, c, cc * 384:(cc + 1) * 384],
                                start=(c == 0), stop=(c == NCH - 1))
                    # engines can't write SBUF at arbitrary base partitions:
                    # evacuate at partition 0, then SBUF->SBUF DMA scatters
                    # the row to its batch partition
                    orow_st = sm.tile([1, M], F32, tag="orow_st", bufs=6)
                    nc.scalar.copy(orow_st.rearrange("p (a b) -> p a b", a=2),
                                   ob[:, :, 0:384])
                    nc.sync.dma_start(out=o_rows[col:col + 1, :], in_=orow_st)

            if stage < 1:
                continue

            # ---------------- phase B: dense chain (feature-major) ----------------
            if stage >= 2:
                _phase_b(nc, ctx, tc, const, pb, tp, bigp, bp, stage, identity,
                         o_rows, fc_wT, w1T, w2T, src_rows, srct_rows, srcT,
                         lnw_cols, lnb_cols, ones_col, ones_row, eps_t, out_h)

    nc.compile()
    return nc


def _phase_b(nc, ctx, tc, const, pb, tp, bigp, bp, stage, identity,
             o_rows, fc_wT, w1T, w2T, src_rows, srct_rows, srcT,
             lnw_cols, lnb_cols, ones_col, ones_row, eps_t, out_h):
        def debug_out(sel):
            out_rows0 = const.tile([bp, D], F32)
            for oc in range(OCH):
                ots0 = pb.tile([128, bp], F32, tag="ot_sb", bufs=2)
                nc.vector.tensor_copy(ots0, sel[:, oc, :])
                pt0 = tp.tile([bp, 128], F32, name="outtp0", tag="tp")
                nc.tensor.transpose(pt0, ots0, identity)
                nc.vector.tensor_copy(out_rows0[:, oc * 128:(oc + 1) * 128], pt0)
            nc.sync.dma_start(out=out_h[:, :], in_=out_rows0)

        # transpose batch-major o rows to feature-major oT
        oT_sb = pb.tile([128, MCH, bp], F32, tag="feat", bufs=2)
        for mc in range(MCH):
            pto = tp.tile([128, 128], F32, name="otp", tag="tp")
            nc.tensor.transpose(pto, o_rows[:, mc * 128:(mc + 1) * 128],
                                identity)
            nc.vector.tensor_copy(oT_sb[:, mc, :], pto)
        if stage == 2:
            debug_out(oT_sb)
            return

        # out2T[j, b] = sum_m fc_w[j, m] oT[m, b]  (+ q residual via transposes)
        fc_psum = bigp.tile([128, MCH, bp], F32, tag="big")
        for jc in range(MCH):
            mms = [(fc_wT[mc][:, jc * 128:(jc + 1) * 128], oT_sb[:, mc, :])
                   for mc in range(MCH)]
            if jc < 2:
                mms.append((src_rows[:, jc * 128:(jc + 1) * 128],
                            identity[:bp, :bp]))
            elif jc >= 4:
                mms.append((srct_rows[:, (jc - 4) * 128:(jc - 3) * 128],
                            identity[:bp, :bp]))
            for q, (l, r) in enumerate(mms):
                nc.tensor.matmul(fc_psum[:, jc, :], lhsT=l, rhs=r,
                                 start=(q == 0), stop=(q == len(mms) - 1))

        # LayerNorm over the feature (partition) dim
        x_sb = pb.tile([128, MCH, bp], F32, tag="feat", bufs=2)
        nc.scalar.copy(x_sb.rearrange("p a b -> p (a b)"),
                       fc_psum.rearrange("p a b -> p (a b)"))
        sq_sb = pb.tile([128, MCH, bp], F32, tag="feat", bufs=2)
        nc.scalar.square(sq_sb.rearrange("p a b -> p (a b)"),
                         fc_psum.rearrange("p a b -> p (a b)"))
        stat = tp.tile([1, 2, bp], F32, tag="tp")
        for mc in range(MCH):
            nc.tensor.matmul(stat[:, 0, :], lhsT=ones_col, rhs=x_sb[:, mc, :],
                             start=(mc == 0), stop=(mc == MCH - 1))
        for mc in range(MCH):
            nc.tensor.matmul(stat[:, 1, :], lhsT=ones_col, rhs=sq_sb[:, mc, :],
                             start=(mc == 0), stop=(mc == MCH - 1))

        mu = pb.tile([1, bp], F32)
        nc.vector.tensor_scalar(mu, stat[:, 0, :], 1.0 / M, None, ALU.mult)
        var = pb.tile([1, bp], F32)
        nc.vector.tensor_scalar(var, stat[:, 1, :], 1.0 / M, None, ALU.mult)
        musq = pb.tile([1, bp], F32)
        nc.vector.tensor_mul(musq, mu, mu)
        nc.vector.tensor_tensor(var, var, musq, ALU.subtract)
        sd = pb.tile([1, bp], F32)
        nc.scalar.activation(sd, var, AF.Sqrt, bias=eps_t[:, 0:1], scale=1.0)
        ab_row = pb.tile([1, 2, bp], F32)
        nc.vector.reciprocal(ab_row[:, 0, :], sd)                  # rstd
        nc.vector.scalar_tensor_tensor(ab_row[:, 1, :], mu, -1.0,
                                       ab_row[:, 0, :], ALU.mult, ALU.mult)
        bc = tp.tile([128, 2, bp], F32, tag="tp")
        nc.tensor.matmul(bc, lhsT=ones_row,
                         rhs=ab_row.rearrange("p a b -> p (a b)"),
                         start=True, stop=True)

        xln = pb.tile([128, MCH, bp], F32, tag="feat", bufs=2)
        for mc in range(MCH):
            nc.vector.tensor_mul(xln[:, mc, :], x_sb[:, mc, :], bc[:, 0, :])
            nc.vector.tensor_add(xln[:, mc, :], xln[:, mc, :], bc[:, 1, :])
            nc.vector.tensor_scalar(xln[:, mc, :], xln[:, mc, :],
                                    lnw_cols[:, mc:mc + 1],
                                    lnb_cols[:, mc:mc + 1], ALU.mult, ALU.add)

        if stage == 3:
            debug_out(xln)
            return

        # agg1: x1T[i, b] = relu(sum_f w1[i, f] catT[f, b])
        x1_psum = bigp.tile([128, MCH, bp], F32, tag="big")
        for ic in range(MCH):
            for fc in range(FCH):
                rhs = xln[:, fc, :] if fc < MCH else srcT[:, fc - MCH, :]
                nc.tensor.matmul(x1_psum[:, ic, :],
                                 lhsT=w1T[fc][:, ic * 128:(ic + 1) * 128],
                                 rhs=rhs, start=(fc == 0), stop=(fc == FCH - 1))
        x1_sb = pb.tile([128, MCH, bp], F32, tag="feat", bufs=2)
        nc.scalar.activation(x1_sb.rearrange("p a b -> p (a b)"),
                             x1_psum.rearrange("p a b -> p (a b)"), AF.Relu)

        # agg2: outT[o, b] = sum_i w2[o, i] x1T[i, b]
        outF = bigp.tile([128, OCH, bp], F32, tag="big")
        for oc in range(OCH):
            for ic in range(MCH):
                nc.tensor.matmul(outF[:, oc, :],
                                 lhsT=w2T[ic][:, oc * 128:(oc + 1) * 128],
                                 rhs=x1_sb[:, ic, :],
                                 start=(ic == 0), stop=(ic == MCH - 1))

        # transpose back to batch-major rows and store
        out_rows = const.tile([bp, D], F32)
        for oc in range(OCH):
            ot_sb = pb.tile([128, bp], F32, tag="ot_sb", bufs=2)
            nc.scalar.copy(ot_sb, outF[:, oc, :])
            pt3 = tp.tile([bp, 128], F32, name="outtp", tag="tp")
            nc.tensor.transpose(pt3, ot_sb, identity)
            nc.vector.tensor_copy(out_rows[:, oc * 128:(oc + 1) * 128], pt3)
        nc.sync.dma_start(out=out_h[:, :], in_=out_rows)


def _shard_inputs(inputs, bpc):
    """Split batch-dim inputs into per-core maps; replicate params."""
    f32 = lambda x: np.ascontiguousarray(np.asarray(x), dtype=np.float32)
    seq = f32(inputs["seq"])
    seq_e = f32(inputs["seq_e"])
    seq_t = f32(inputs["seq_t"])
    src = f32(inputs["src"])
    src_t = f32(inputs["src_t"])
    mask = np.ascontiguousarray(np.asarray(inputs["mask"])).astype(np.uint8)
    params = {
        "shared_attn": f32(inputs["shared_attn"]),
        "fc_w": f32(inputs["fc_w"]),
        "ln_w": f32(inputs["ln_w"]),
        "ln_b": f32(inputs["ln_b"]),
        "agg_fc_w1": f32(inputs["agg_fc_w1"]),
        "agg_fc_w2": f32(inputs["agg_fc_w2"]),
    }
    in_maps = []
    for i in range(NCORES):
        sl = slice(i * bpc, (i + 1) * bpc)
        in_maps.append({
            "seq": seq[sl], "seq_e": seq_e[sl], "seq_t": seq_t[sl],
            "src": src[sl], "src_t": src_t[sl], "mask": mask[sl],
            **params,
        })
    return in_maps


def kernel(**inputs):
    bpc = B // NCORES
    nc = build_bass(bpc=bpc)
    in_maps = _shard_inputs(inputs, bpc)
    res = run_bass_kernel_spmd(nc, in_maps, core_ids=list(range(NCORES)))
    output = np.concatenate([r["out"] for r in res.results], axis=0)
    attn_w = np.concatenate([r["attn_w"] for r in res.results], axis=0)
    return output, attn_w

